# revision 1
# baseline (speedup 1.0000x reference)
"""Trainium2 Bass kernel for a pre-norm transformer encoder layer.

Problem shapes (hardcoded): x [4, 2048, 1024], 16 heads x 64, d_ff 4096.

Sharding: 8 cores = (batch b, query-half q): core c -> b = c//2, q-rows
[q*1024, (q+1)*1024). Each core computes K/V for its batch's full 2048
tokens (12% duplicated projection work) and everything else for its own
1024 query tokens. No collectives.

On-chip layout is feature-major throughout: activations are stored
transposed ([d on partitions, tokens on free]), which every matmul here
wants (contraction over partitions); the host passes x pre-transposed
and transposes the output back. LayerNorm statistics are computed with
ones-vector matmuls on the tensor engine (partition-dim reductions) and
per-token rows are broadcast across partitions with a rank-1 matmul.
LN gain/bias are folded into the following projection weights on the
host. Softmax runs without max-subtraction (scores here are O(8), exp
is safe in fp32); the attention mask folds into the Exp activation's
per-partition bias; the softmax denominator comes free as a 65th
ones-column appended to V in the P@V matmul.

Matmul inputs are bf16 (fp32 PSUM accumulation); the residual stream
stays fp32.

SBUF is tight (203KB/partition), so the big tensors share tag "chains"
(same pool tag -> same slots, the Tile scheduler serializes reuse):
  chA: h1 -> r1         chB: xkv -> KT -> g_qt      chC: xq -> V -> h2
  chD: h1q -> OT        chE: QT
Weights stream through one 2-buffered 8KB tag in host-pre-tiled layouts
(long DMA lines). PSUM: tag "acc" (4 banks: stats/proj/scores/ffn),
"bc" (2: broadcasts + softmax recip-broadcast), "o" (2: PV accum).
"""

import numpy as np
import ml_dtypes

import concourse.bass as bass
import concourse.bacc as bacc
import concourse.tile as tile
from concourse import mybir
from concourse.bass_utils import run_bass_kernel_spmd

AF = mybir.ActivationFunctionType
OP = mybir.AluOpType
BF = mybir.dt.bfloat16
F32 = mybir.dt.float32

P = 128
D = 1024
DC = D // P            # 8 d-chunks
TKV = 2048             # kv tokens per core (= S of its batch)
TQ = 1024              # query tokens per core
NKV = TKV // 512       # 4
NQ = TQ // 512         # 2
KC = TKV // P          # 16 key chunks
H = 16
DK = 64
F = 4096
FC = F // P            # 32 ff chunks
EPS = 1e-5
N = 512                # matmul moving free dim / psum bank (fp32)


def _ln_feature(nc, pool, ps, src, ntok, sq_dt, ones_col, ones_row,
                eps_bias, RS, M2):
    """LayerNorm over the partition (d) axis of feature-major src
    [P, DC, ntok]: per 512-token slice compute sum / sum-of-squares via
    ones-matmuls, derive rs = 1/sqrt(var+eps) and m2 = -mu*rs rows, and
    broadcast them across partitions into RS / M2."""
    for t in range(ntok // N):
        sum_ps = ps.tile([P, N], F32, tag="acc", bufs=4)
        sumsq_ps = ps.tile([P, N], F32, tag="acc", bufs=4)
        for c in range(DC):
            sl = src[:, c, t * N:(t + 1) * N]
            nc.tensor.matmul(sum_ps[0:1, :], ones_col[:], sl,
                             start=(c == 0), stop=(c == DC - 1))
            sq = pool.tile([P, N], sq_dt, tag="sq", bufs=3)
            nc.scalar.activation(sq[:], sl, AF.Square)
            nc.tensor.matmul(sumsq_ps[0:1, :], ones_col[:], sq[:],
                             start=(c == 0), stop=(c == DC - 1))
        mu = pool.tile([1, N], F32, tag="lnrow", bufs=4)
        nc.vector.tensor_scalar_mul(mu[:], sum_ps[0:1, :], 1.0 / D)
        musq = pool.tile([1, N], F32, tag="lnrow", bufs=4)
        nc.vector.tensor_mul(musq[:], mu[:], mu[:])
        var = pool.tile([1, N], F32, tag="lnrow", bufs=4)
        nc.vector.scalar_tensor_tensor(
            out=var[:], in0=sumsq_ps[0:1, :], scalar=1.0 / D, in1=musq[:],
            op0=OP.mult, op1=OP.subtract)
        std = pool.tile([1, N], F32, tag="lnrow", bufs=4)
        nc.scalar.activation(std[:], var[:], AF.Sqrt, bias=eps_bias,
                             scale=1.0)
        rs = pool.tile([1, N], F32, tag="lnrow", bufs=4)
        nc.vector.reciprocal(rs[:], std[:])
        m2 = pool.tile([1, N], F32, tag="lnrow", bufs=4)
        nc.vector.scalar_tensor_tensor(
            out=m2[:], in0=mu[:], scalar=-1.0, in1=rs[:],
            op0=OP.mult, op1=OP.mult)
        bps = ps.tile([P, N], F32, tag="bc", bufs=2)
        nc.tensor.matmul(bps[:, :], ones_row[0:1, :], rs[0:1, :])
        nc.vector.tensor_copy(RS[:, t * N:(t + 1) * N], bps[:, :])
        bps2 = ps.tile([P, N], F32, tag="bc", bufs=2)
        nc.tensor.matmul(bps2[:, :], ones_row[0:1, :], m2[0:1, :])
        nc.vector.tensor_copy(M2[:, t * N:(t + 1) * N], bps2[:, :])


def build_body(tc, d, upto="full"):
    nc = tc.nc
    with tc.tile_pool(name="sb", bufs=1) as pool, \
         tc.tile_pool(name="ps", bufs=2, space="PSUM") as ps:
        _build_inner(nc, pool, ps, d, upto)


def _build_inner(nc, pool, ps, d, upto="full"):
    def anchor(t):
        """DMA a slice out to keep the truncated pipeline live."""
        nc.gpsimd.dma_start(out=d["outT"][0:P, 0:N], in_=t)
    # --- constants ---
    ones_col_bf = pool.tile([P, 1], BF, tag="c_onesb")
    nc.vector.memset(ones_col_bf[:], 1.0)
    ones_col_f32 = pool.tile([P, 1], F32, tag="c_onesf")
    nc.vector.memset(ones_col_f32[:], 1.0)
    ones_row_f32 = pool.tile([1, P], F32, tag="c_onesr")
    nc.vector.memset(ones_row_f32[:], 1.0)
    eps_bias = pool.tile([1, 1], F32, tag="c_eps")
    nc.vector.memset(eps_bias[:], EPS)

    bias_c = {}
    for nm, w in [("bq_c", DC), ("bk_c", DC), ("bo_c", DC), ("b2_c", DC),
                  ("b1_c", FC), ("maskb_c", KC)]:
        t = pool.tile([P, w], F32, tag=f"c_{nm}")
        nc.sync.dma_start(t[:], d[nm][:])
        bias_c[nm] = t
    bq_sb, bk_sb, bo_sb = bias_c["bq_c"], bias_c["bk_c"], bias_c["bo_c"]
    b2_sb, b1_sb, mask_sb = bias_c["b2_c"], bias_c["b1_c"], bias_c["maskb_c"]
    bv_sb = pool.tile([P, D], F32, tag="c_bv")
    bv_ap = d["bv"]
    nc.gpsimd.dma_start(
        out=bv_sb[:],
        in_=bass.AP(tensor=bv_ap.tensor, offset=bv_ap.offset,
                    ap=[[0, P]] + [list(a) for a in bv_ap.ap]))

    # =====================================================================
    # Stage A: LN1 over kv tokens (2048) and q tokens (1024), apply
    # =====================================================================
    xkv = pool.tile([P, DC, TKV], BF, tag="chB")
    nc.sync.dma_start(xkv[:], d["xkvT"].rearrange("(c p) t -> p c t", p=P))
    xq_sb = pool.tile([P, DC, TQ], F32, tag="chC")
    nc.sync.dma_start(xq_sb[:], d["xqT"].rearrange("(c p) t -> p c t", p=P))

    if upto == "load":
        anchor(xkv[:, 0, 0:N])
        anchor(xq_sb[:, 0, 0:N])
        return
    RS1 = pool.tile([P, TKV], BF, tag="RS1")
    M21 = pool.tile([P, TKV], BF, tag="M21")
    _ln_feature(nc, pool, ps, xkv, TKV, BF, ones_col_bf, ones_row_f32,
                eps_bias, RS1, M21)
    if upto == "lnkv":
        anchor(RS1[:, 0:N])
        anchor(M21[:, 0:N])
        return
    RSq = pool.tile([P, TQ], F32, tag="RSx")
    M2q = pool.tile([P, TQ], F32, tag="M2x")
    _ln_feature(nc, pool, ps, xq_sb, TQ, F32, ones_col_f32, ones_row_f32,
                eps_bias, RSq, M2q)

    # apply LN (in-place second op) -> h1T, h1qT (bf16)
    h1 = pool.tile([P, DC, TKV], BF, tag="chA")
    h1q = pool.tile([P, DC, TQ], BF, tag="chD")
    for c in range(DC):
        nc.vector.tensor_mul(h1[:, c, :], xkv[:, c, :], RS1[:])
        nc.vector.tensor_add(h1[:, c, :], h1[:, c, :], M21[:])
        nc.vector.tensor_mul(h1q[:, c, :], xq_sb[:, c, :], RSq[:])
        nc.vector.tensor_add(h1q[:, c, :], h1q[:, c, :], M2q[:])

    if upto == "ln":
        anchor(h1[:, 0, 0:N])
        return
    # =====================================================================
    # Stage B: Q/K/V projections (weights streamed, host-pre-tiled)
    # =====================================================================
    KT = pool.tile([P, DC, TKV], BF, tag="chB")     # K^T feature-major
    QT = pool.tile([P, DC, TQ], BF, tag="chE")      # Q^T feature-major
    V = pool.tile([P, KC, H, DK + 1], BF, tag="chC")  # token-major V + ones
    nc.vector.memset(V[:, :, :, DK:DK + 1], 1.0)

    for ocg in range(2):
        wkg = pool.tile([P, DC, 512], BF, tag="w", bufs=2)
        nc.sync.dma_start(wkg[:], d["wk"][:, ocg, :, :])
        for oci in range(4):
            oc = ocg * 4 + oci
            for t in range(NKV):
                pp = ps.tile([P, N], F32, tag="acc", bufs=4)
                for c in range(DC):
                    nc.tensor.matmul(pp[:], wkg[:, c, oci * P:(oci + 1) * P],
                                     h1[:, c, t * N:(t + 1) * N],
                                     start=(c == 0), stop=(c == DC - 1))
                nc.scalar.activation(KT[:, oc, t * N:(t + 1) * N], pp[:],
                                     AF.Identity, bias=bk_sb[:, oc:oc + 1],
                                     scale=1.0)
    for ocg in range(2):
        wqg = pool.tile([P, DC, 512], BF, tag="w", bufs=2)
        nc.sync.dma_start(wqg[:], d["wq"][:, ocg, :, :])
        for oci in range(4):
            oc = ocg * 4 + oci
            for t in range(NQ):
                pp = ps.tile([P, N], F32, tag="acc", bufs=4)
                for c in range(DC):
                    nc.tensor.matmul(pp[:], wqg[:, c, oci * P:(oci + 1) * P],
                                     h1q[:, c, t * N:(t + 1) * N],
                                     start=(c == 0), stop=(c == DC - 1))
                nc.scalar.activation(QT[:, oc, t * N:(t + 1) * N], pp[:],
                                     AF.Identity, bias=bq_sb[:, oc:oc + 1],
                                     scale=1.0)
    for os_ in range(2):
        wvg = pool.tile([P, DC, 512], BF, tag="w", bufs=2)
        nc.sync.dma_start(wvg[:], d["wv"][:, os_, :, :])
        for kc in range(KC):
            pp = ps.tile([P, N], F32, tag="acc", bufs=4)
            for c in range(DC):
                nc.tensor.matmul(pp[:], h1[:, c, kc * P:(kc + 1) * P],
                                 wvg[:, c, :],
                                 start=(c == 0), stop=(c == DC - 1))
            nc.vector.tensor_add(
                V[:, kc, os_ * 8:(os_ + 1) * 8, 0:DK],
                pp[:].rearrange("p (h k) -> p h k", h=8),
                bv_sb[:, os_ * N:(os_ + 1) * N].rearrange(
                    "p (h k) -> p h k", h=8))

    if upto == "proj":
        anchor(KT[:, 0, 0:N])
        anchor(QT[:, 0, 0:N])
        anchor(V[:, 0, :, :].rearrange("p a b -> p (a b)")[:, 0:N])
        return
    # =====================================================================
    # Stage C: attention -> O^T feature-major (bf16)
    # =====================================================================
    OT = pool.tile([P, DC, TQ], BF, tag="chD")
    for h in range(H):
        hp = (h % 2) * DK
        hc = h // 2
        for qt in range(NQ):
            q_sl = QT[hp:hp + DK, hc, qt * N:(qt + 1) * N]
            o_ps = ps.tile([P, N], F32, tag="o", bufs=2)
            for kc in range(KC):
                s_ps = ps.tile([P, N], F32, tag="acc", bufs=4)
                nc.tensor.matmul(
                    s_ps[:], KT[hp:hp + DK, hc, kc * P:(kc + 1) * P], q_sl)
                e = pool.tile([P, N], BF, tag="e", bufs=4)
                nc.scalar.activation(e[:], s_ps[:], AF.Exp,
                                     bias=mask_sb[:, kc:kc + 1], scale=0.125)
                nc.tensor.matmul(o_ps[0:DK + 1, :], V[:, kc, h, :], e[:],
                                 start=(kc == 0), stop=(kc == KC - 1))
            recip = pool.tile([1, N], F32, tag="recip", bufs=2)
            nc.vector.reciprocal(recip[:], o_ps[DK:DK + 1, :])
            r_ps = ps.tile([P, N], F32, tag="bc", bufs=2)
            nc.tensor.matmul(r_ps[0:DK, :], ones_row_f32[0:1, 0:DK], recip[:])
            rbc = pool.tile([DK, N], F32, tag="rbc", bufs=2)
            nc.vector.tensor_copy(rbc[:], r_ps[0:DK, :])
            nc.vector.tensor_mul(OT[hp:hp + DK, hc, qt * N:(qt + 1) * N],
                                 o_ps[0:DK, :], rbc[:])

    if upto == "attn":
        anchor(OT[:, 0, 0:N])
        return
    # =====================================================================
    # Stage D: out projection + residual -> r1 (fp32)
    # =====================================================================
    r1 = pool.tile([P, DC, TQ], F32, tag="chA")
    for ocg in range(2):
        wog = pool.tile([P, DC, 512], BF, tag="w", bufs=2)
        nc.sync.dma_start(wog[:], d["wo"][:, ocg, :, :])
        for oci in range(4):
            oc = ocg * 4 + oci
            xq2 = pool.tile([P, TQ], F32, tag="xq2", bufs=2)
            nc.sync.dma_start(
                xq2[:],
                d["xqT"].rearrange("(c p) t -> p c t", p=P)[:, oc, :])
            for qt in range(NQ):
                pp = ps.tile([P, N], F32, tag="acc", bufs=4)
                for c in range(DC):
                    nc.tensor.matmul(pp[:], wog[:, c, oci * P:(oci + 1) * P],
                                     OT[:, c, qt * N:(qt + 1) * N],
                                     start=(c == 0), stop=(c == DC - 1))
                nc.vector.scalar_tensor_tensor(
                    out=r1[:, oc, qt * N:(qt + 1) * N], in0=pp[:],
                    scalar=bo_sb[:, oc:oc + 1],
                    in1=xq2[:, qt * N:(qt + 1) * N],
                    op0=OP.add, op1=OP.add)

    # =====================================================================
    # Stage E: LN2 -> h2 (bf16)
    # =====================================================================
    RS2 = pool.tile([P, TQ], F32, tag="RSx")
    M22 = pool.tile([P, TQ], F32, tag="M2x")
    _ln_feature(nc, pool, ps, r1, TQ, F32, ones_col_f32, ones_row_f32,
                eps_bias, RS2, M22)
    h2 = pool.tile([P, DC, TQ], BF, tag="chC")
    for c in range(DC):
        nc.vector.tensor_mul(h2[:, c, :], r1[:, c, :], RS2[:])
        nc.vector.tensor_add(h2[:, c, :], h2[:, c, :], M22[:])

    if upto == "ln2":
        anchor(h2[:, 0, 0:N])
        return
    # =====================================================================
    # Stage F: FFN, one query-tile (512 tokens) at a time
    # =====================================================================
    for qt in range(NQ):
        g = pool.tile([P, FC, N], BF, tag="chB")   # gelu(fc1) for this qt
        for fcg in range(8):
            w1g = pool.tile([P, DC, 512], BF, tag="w", bufs=2)
            nc.sync.dma_start(w1g[:], d["w1"][:, fcg, :, :])
            for fi in range(4):
                fc = fcg * 4 + fi
                pp = ps.tile([P, N], F32, tag="acc", bufs=4)
                for c in range(DC):
                    nc.tensor.matmul(pp[:], w1g[:, c, fi * P:(fi + 1) * P],
                                     h2[:, c, qt * N:(qt + 1) * N],
                                     start=(c == 0), stop=(c == DC - 1))
                nc.scalar.activation(g[:, fc, :], pp[:], AF.Gelu,
                                     bias=b1_sb[:, fc:fc + 1], scale=1.0)
        for oc in range(DC):
            w2g = pool.tile([P, FC, P], BF, tag="w", bufs=2)
            nc.sync.dma_start(w2g[:], d["w2"][:, oc, :, :])
            pp = ps.tile([P, N], F32, tag="acc", bufs=4)
            for f in range(FC):
                nc.tensor.matmul(pp[:], w2g[:, f, :], g[:, f, :],
                                 start=(f == 0), stop=(f == FC - 1))
            ot = pool.tile([P, N], F32, tag="out", bufs=2)
            nc.vector.scalar_tensor_tensor(
                out=ot[:], in0=pp[:], scalar=b2_sb[:, oc:oc + 1],
                in1=r1[:, oc, qt * N:(qt + 1) * N],
                op0=OP.add, op1=OP.add)
            nc.sync.dma_start(
                d["outT"][oc * P:(oc + 1) * P, qt * N:(qt + 1) * N], ot[:])


def build_program(repeat=1, upto="full"):
    nc = bacc.Bacc("TRN2", target_bir_lowering=False)
    d = {}
    d["xkvT"] = nc.declare_dram_parameter("xkvT", [D, TKV], BF, isOutput=False)[:]
    d["xqT"] = nc.declare_dram_parameter("xqT", [D, TQ], F32, isOutput=False)[:]
    for w, sh in [("wq", [P, 2, DC, 512]), ("wk", [P, 2, DC, 512]),
                  ("wv", [P, 2, DC, 512]), ("wo", [P, 2, DC, 512]),
                  ("w1", [P, 8, DC, 512]), ("w2", [P, DC, FC, P])]:
        d[w] = nc.declare_dram_parameter(w, sh, BF, isOutput=False)[:]
    for b, sh in [("bq_c", [P, DC]), ("bk_c", [P, DC]), ("bo_c", [P, DC]),
                  ("b2_c", [P, DC]), ("b1_c", [P, FC]), ("maskb_c", [P, KC]),
                  ("bv", [D])]:
        d[b] = nc.declare_dram_parameter(b, sh, F32, isOutput=False)[:]
    d["outT"] = nc.declare_dram_parameter("outT", [D, TQ], F32, isOutput=True)[:]

    with tile.TileContext(nc) as tc:
        for _ in range(repeat):
            build_body(tc, d, upto)
    nc.compile()
    return nc


_PROG_CACHE = {}


def _get_prog(repeat=1, upto="full"):
    key = (repeat, upto)
    if key not in _PROG_CACHE:
        _PROG_CACHE[key] = build_program(repeat, upto)
    return _PROG_CACHE[key]


def _tile_w(Wm, ngrp, gsz):
    """[1024-in, ngrp*gsz-out] -> [128, ngrp, in-chunks, gsz] host tiling."""
    ic = Wm.shape[0] // P
    return np.ascontiguousarray(
        Wm.reshape(ic, P, ngrp, gsz).transpose(1, 2, 0, 3))


def make_in_maps(x, mask, Wq, bq, Wk, bk, Wv, bv, Wo, bo, W1, b1, W2, b2,
                 ln1_g, ln1_b, ln2_g, ln2_b):
    x = np.asarray(x, np.float32)
    bf = ml_dtypes.bfloat16

    def fold(Wm, bm):
        Wf = np.asarray(ln1_g, np.float64)[:, None] * np.asarray(Wm, np.float64)
        bfold = (np.asarray(ln1_b, np.float64) @ np.asarray(Wm, np.float64)
                 + np.asarray(bm, np.float64))
        return Wf.astype(np.float32), bfold.astype(np.float32)

    wq_f, bq_f = fold(Wq, bq)
    wk_f, bk_f = fold(Wk, bk)
    wv_f, bv_f = fold(Wv, bv)
    w1_f = (np.asarray(ln2_g, np.float64)[:, None]
            * np.asarray(W1, np.float64)).astype(np.float32)
    b1_f = (np.asarray(ln2_b, np.float64) @ np.asarray(W1, np.float64)
            + np.asarray(b1, np.float64)).astype(np.float32)

    def cols(v, k):
        return np.ascontiguousarray(np.asarray(v, np.float32).reshape(k, P).T)

    shared = {
        "wq": _tile_w(wq_f, 2, 512).astype(bf),
        "wk": _tile_w(wk_f, 2, 512).astype(bf),
        "wv": _tile_w(wv_f, 2, 512).astype(bf),
        "wo": _tile_w(np.asarray(Wo, np.float32), 2, 512).astype(bf),
        "w1": _tile_w(w1_f, 8, 512).astype(bf),
        "w2": _tile_w(np.asarray(W2, np.float32), 8, 128).astype(bf),
        "bq_c": cols(bq_f, DC), "bk_c": cols(bk_f, DC),
        "bo_c": cols(np.asarray(bo, np.float32), DC),
        "b2_c": cols(np.asarray(b2, np.float32), DC),
        "b1_c": cols(b1_f, FC),
        "bv": bv_f,
    }
    in_maps = []
    mask = np.asarray(mask)
    for c in range(8):
        b, q = c // 2, c % 2
        xT = np.ascontiguousarray(x[b].T)                     # [D, TKV]
        mb = np.where(mask[b, 0, 0, :] == 0, np.float32(-1e9),
                      np.float32(0.0)).astype(np.float32)
        m = dict(shared)
        m["xkvT"] = xT.astype(bf)
        m["xqT"] = np.ascontiguousarray(xT[:, q * TQ:(q + 1) * TQ])
        m["maskb_c"] = np.ascontiguousarray(mb.reshape(KC, P).T)
        in_maps.append(m)
    return in_maps


def gather_out(results):
    out = np.empty((4, 2048, 1024), np.float32)
    for c in range(8):
        b, q = c // 2, c % 2
        out[b, q * TQ:(q + 1) * TQ, :] = results[c]["outT"].T
    return out


def kernel(**inputs):
    nc = _get_prog(1)
    in_maps = make_in_maps(**inputs)
    res = run_bass_kernel_spmd(nc, in_maps, list(range(8)))
    return gather_out(res.results)



# revision 14
# speedup vs baseline: 628.9069x; 628.9069x over previous
"""Trainium2 Bass kernel for a pre-norm transformer encoder layer.

Problem shapes (hardcoded): x [4, 2048, 1024], 16 heads x 64, d_ff 4096.

Sharding: 8 cores = (batch b, query-half q): core c -> b = c//2, q-rows
[q*1024, (q+1)*1024). Each core computes K/V for its batch's full 2048
tokens (12% duplicated projection work) and everything else for its own
1024 query tokens. No collectives.

On-chip layout is feature-major throughout: activations are stored
transposed ([d on partitions, tokens on free]), which every matmul here
wants (contraction over partitions); the host passes x pre-transposed
and transposes the output back. LayerNorm statistics are computed with
ones-vector matmuls on the tensor engine (partition-dim reductions) and
per-token rows are broadcast across partitions with a rank-1 matmul;
rs = rsqrt(var+eps) comes from one ACT Rsqrt. LN gain/bias are folded
into the following projection weights on the host. Softmax runs without
max-subtraction (scores here are O(8), exp is safe in fp32); the
attention mask folds into the Exp activation's per-partition bias; the
softmax denominator comes free as a 65th ones-column appended to V in
the P@V matmul, normalized once per head over all 1024 queries.

PSUM scheme: two tags of [128, 1024] f32 x 2 bufs = 8 banks. Matmuls
write 512-wide halves (bank limit); ACT/DVE consume the full 1024-wide
tile in one instruction, halving ACT/DVE instruction counts and per-
instruction overhead. Matmul pairs writing both halves share their
stationary operand back-to-back.

Matmul inputs are bf16 (fp32 PSUM accumulation); the residual stream
stays fp32.

SBUF is tight (203KB/partition), so the big tensors share tag "chains"
(same pool tag -> same slots, the Tile scheduler serializes reuse):
  chA: h1 -> r1         chB: xkv -> KT -> g_qt      chC: xq -> V -> h2
  chD: h1q -> OT        chE: QT
Weights stream through one 2-buffered 8KB tag in host-pre-tiled layouts
(long DMA lines).
"""

import numpy as np
import ml_dtypes

import concourse.bass as bass
import concourse.bacc as bacc
import concourse.tile as tile
from concourse import mybir
from concourse.bass_utils import run_bass_kernel_spmd

AF = mybir.ActivationFunctionType
OP = mybir.AluOpType
BF = mybir.dt.bfloat16
F32 = mybir.dt.float32

P = 128
D = 1024
DC = D // P            # 8 d-chunks
TKV = 2048             # kv tokens per core (= S of its batch)
TQ = 1024              # query tokens per core
NKV = TKV // 512       # 4
NQ = TQ // 512         # 2
KC = TKV // P          # 16 key chunks
H = 16
DK = 64
F = 4096
FC = F // P            # 32 ff chunks
EPS = 1e-5
N = 512                # matmul moving free dim / psum bank (fp32)
N2 = 2 * N             # wide tile: 2 psum banks


def _ln_feature(nc, pool, ps, src, ntok, ones_sum, ones_bf, ones_row,
                eps_bias, RS, M2):
    """LayerNorm over the partition (d) axis of feature-major src
    [P, DC, ntok]: per 1024-token slice compute sum / sum-of-squares via
    ones-matmuls (512-wide halves), derive rs = 1/sqrt(var+eps) and
    m2 = -mu*rs rows, and broadcast them across partitions into RS/M2.
    ones_sum matches src dtype; squares are bf16 (ones_bf)."""
    for t in range(ntok // N2):
        sum_ps = ps.tile([P, N2], F32, tag="psA", bufs=2)
        sumsq_ps = ps.tile([P, N2], F32, tag="psB", bufs=2)
        for c in range(DC):
            sl = src[:, c, t * N2:(t + 1) * N2]
            nc.tensor.matmul(sum_ps[0:1, 0:N], ones_sum[:], sl[:, 0:N],
                             start=(c == 0), stop=(c == DC - 1))
            nc.tensor.matmul(sum_ps[0:1, N:N2], ones_sum[:], sl[:, N:N2],
                             start=(c == 0), stop=(c == DC - 1))
            sq = pool.tile([P, N2], BF, tag="sq", bufs=3)
            nc.scalar.activation(sq[:], sl, AF.Square)
            nc.tensor.matmul(sumsq_ps[0:1, 0:N], ones_bf[:], sq[:, 0:N],
                             start=(c == 0), stop=(c == DC - 1))
            nc.tensor.matmul(sumsq_ps[0:1, N:N2], ones_bf[:], sq[:, N:N2],
                             start=(c == 0), stop=(c == DC - 1))
        mu = pool.tile([1, N2], F32, tag="lnrow", bufs=4)
        nc.vector.tensor_scalar_mul(mu[:], sum_ps[0:1, :], 1.0 / D)
        musq = pool.tile([1, N2], F32, tag="lnrow", bufs=4)
        nc.vector.tensor_mul(musq[:], mu[:], mu[:])
        var = pool.tile([1, N2], F32, tag="lnrow", bufs=4)
        nc.vector.scalar_tensor_tensor(
            out=var[:], in0=sumsq_ps[0:1, :], scalar=1.0 / D, in1=musq[:],
            op0=OP.mult, op1=OP.subtract)
        std = pool.tile([1, N2], F32, tag="lnrow", bufs=4)
        nc.scalar.activation(std[:], var[:], AF.Sqrt, bias=eps_bias,
                             scale=1.0)
        rs = pool.tile([1, N2], F32, tag="lnrow", bufs=4)
        nc.vector.reciprocal(rs[:], std[:])
        m2 = pool.tile([1, N2], F32, tag="lnrow", bufs=4)
        nc.vector.scalar_tensor_tensor(
            out=m2[:], in0=mu[:], scalar=-1.0, in1=rs[:],
            op0=OP.mult, op1=OP.mult)
        bps = ps.tile([P, N2], F32, tag="psA", bufs=2)
        nc.tensor.matmul(bps[:, 0:N], ones_row[0:1, :], rs[0:1, 0:N])
        nc.tensor.matmul(bps[:, N:N2], ones_row[0:1, :], rs[0:1, N:N2])
        nc.vector.tensor_copy(RS[:, t * N2:(t + 1) * N2], bps[:, :])
        bps2 = ps.tile([P, N2], F32, tag="psB", bufs=2)
        nc.tensor.matmul(bps2[:, 0:N], ones_row[0:1, :], m2[0:1, 0:N])
        nc.tensor.matmul(bps2[:, N:N2], ones_row[0:1, :], m2[0:1, N:N2])
        nc.vector.tensor_copy(M2[:, t * N2:(t + 1) * N2], bps2[:, :])


def build_body(tc, d, upto="full"):
    nc = tc.nc
    with tc.tile_pool(name="sb", bufs=1) as pool, \
         tc.tile_pool(name="ps", bufs=2, space="PSUM") as ps:
        _build_inner(nc, pool, ps, d, upto)


def _build_inner(nc, pool, ps, d, upto="full"):
    def anchor(t):
        """DMA a slice out to keep the truncated pipeline live."""
        nc.gpsimd.dma_start(out=d["outT"][0:P, 0:N], in_=t)
    # --- constants ---
    ones_col_bf = pool.tile([P, 1], BF, tag="c_onesb")
    nc.vector.memset(ones_col_bf[:], 1.0)
    ones_col_f32 = pool.tile([P, 1], F32, tag="c_onesf")
    nc.vector.memset(ones_col_f32[:], 1.0)
    ones_row_f32 = pool.tile([1, P], F32, tag="c_onesr")
    nc.vector.memset(ones_row_f32[:], 1.0)
    eps_bias = pool.tile([1, 1], F32, tag="c_eps")
    nc.vector.memset(eps_bias[:], EPS)

    bias_c = {}
    for nm, w in [("bq_c", DC), ("bk_c", DC), ("bo_c", DC), ("b2_c", DC),
                  ("b1_c", FC), ("maskb_c", KC)]:
        t = pool.tile([P, w], F32, tag=f"c_{nm}")
        nc.sync.dma_start(t[:], d[nm][:])
        bias_c[nm] = t
    bq_sb, bk_sb, bo_sb = bias_c["bq_c"], bias_c["bk_c"], bias_c["bo_c"]
    b2_sb, b1_sb, mask_sb = bias_c["b2_c"], bias_c["b1_c"], bias_c["maskb_c"]
    bv_sb = pool.tile([P, D], F32, tag="c_bv")
    nc.sync.dma_start(bv_sb[:], d["bvb"][:])

    # =====================================================================
    # Stage A: LN1 over kv tokens (2048) and q tokens (1024), apply
    # =====================================================================
    xkv = pool.tile([P, DC, TKV], BF, tag="chB")
    nc.sync.dma_start(xkv[:], d["xkvT"].rearrange("(c p) t -> p c t", p=P))
    xq_sb = pool.tile([P, DC, TQ], F32, tag="chC")
    nc.sync.dma_start(xq_sb[:], d["xqT"].rearrange("(c p) t -> p c t", p=P))

    if upto == "load":
        anchor(xkv[:, 0, 0:N])
        anchor(xq_sb[:, 0, 0:N])
        return
    RS1 = pool.tile([P, TKV], BF, tag="RS1")
    M21 = pool.tile([P, TKV], BF, tag="M21")
    _ln_feature(nc, pool, ps, xkv, TKV, ones_col_bf, ones_col_bf,
                ones_row_f32, eps_bias, RS1, M21)
    if upto == "lnkv":
        anchor(RS1[:, 0:N])
        anchor(M21[:, 0:N])
        return
    RSq = pool.tile([P, TQ], F32, tag="RSx")
    M2q = pool.tile([P, TQ], F32, tag="M2x")
    _ln_feature(nc, pool, ps, xq_sb, TQ, ones_col_f32, ones_col_bf,
                ones_row_f32, eps_bias, RSq, M2q)

    # apply LN (in-place second op) -> h1T, h1qT (bf16)
    h1 = pool.tile([P, DC, TKV], BF, tag="chA")
    h1q = pool.tile([P, DC, TQ], BF, tag="chD")
    for c in range(DC):
        nc.vector.tensor_mul(h1[:, c, :], xkv[:, c, :], RS1[:])
        nc.vector.tensor_add(h1[:, c, :], h1[:, c, :], M21[:])
        nc.vector.tensor_mul(h1q[:, c, :], xq_sb[:, c, :], RSq[:])
        nc.vector.tensor_add(h1q[:, c, :], h1q[:, c, :], M2q[:])

    if upto == "ln":
        anchor(h1[:, 0, 0:N])
        return
    # =====================================================================
    # Stage B: Q/K/V projections (weights streamed, host-pre-tiled)
    # =====================================================================
    KT = pool.tile([P, DC, TKV], BF, tag="chB")     # K^T feature-major
    QT = pool.tile([P, DC, TQ], BF, tag="chE")      # Q^T feature-major
    V = pool.tile([P, KC, H, DK + 1], BF, tag="chC")  # token-major V + ones
    nc.vector.memset(V[:, :, :, DK:DK + 1], 1.0)

    for ocg in range(2):
        wkg = pool.tile([P, DC, 512], BF, tag="w", bufs=2)
        nc.sync.dma_start(wkg[:], d["wk"][:, ocg, :, :])
        for oci in range(4):
            oc = ocg * 4 + oci
            for tp in range(NKV // 2):
                pp = ps.tile([P, N2], F32, tag="psA", bufs=2)
                for c in range(DC):
                    w_sl = wkg[:, c, oci * P:(oci + 1) * P]
                    nc.tensor.matmul(pp[:, 0:N], w_sl,
                                     h1[:, c, tp * N2:tp * N2 + N],
                                     start=(c == 0), stop=(c == DC - 1))
                    nc.tensor.matmul(pp[:, N:N2], w_sl,
                                     h1[:, c, tp * N2 + N:(tp + 1) * N2],
                                     start=(c == 0), stop=(c == DC - 1))
                nc.scalar.activation(KT[:, oc, tp * N2:(tp + 1) * N2], pp[:],
                                     AF.Identity, bias=bk_sb[:, oc:oc + 1],
                                     scale=1.0)
    for ocg in range(2):
        wqg = pool.tile([P, DC, 512], BF, tag="w", bufs=2)
        nc.sync.dma_start(wqg[:], d["wq"][:, ocg, :, :])
        for oci in range(4):
            oc = ocg * 4 + oci
            pp = ps.tile([P, N2], F32, tag="psA", bufs=2)
            for c in range(DC):
                w_sl = wqg[:, c, oci * P:(oci + 1) * P]
                nc.tensor.matmul(pp[:, 0:N], w_sl, h1q[:, c, 0:N],
                                 start=(c == 0), stop=(c == DC - 1))
                nc.tensor.matmul(pp[:, N:N2], w_sl, h1q[:, c, N:N2],
                                 start=(c == 0), stop=(c == DC - 1))
            nc.scalar.activation(QT[:, oc, :], pp[:],
                                 AF.Identity, bias=bq_sb[:, oc:oc + 1],
                                 scale=1.0)
    for os_ in range(2):
        wvg = pool.tile([P, DC, 512], BF, tag="w", bufs=2)
        nc.sync.dma_start(wvg[:], d["wv"][:, os_, :, :])
        for kcp in range(KC // 2):
            pp = ps.tile([P, N2], F32, tag="psB", bufs=2)
            for c in range(DC):
                nc.tensor.matmul(
                    pp[:, 0:N], h1[:, c, (2 * kcp) * P:(2 * kcp + 1) * P],
                    wvg[:, c, :], start=(c == 0), stop=(c == DC - 1))
                nc.tensor.matmul(
                    pp[:, N:N2], h1[:, c, (2 * kcp + 1) * P:(2 * kcp + 2) * P],
                    wvg[:, c, :], start=(c == 0), stop=(c == DC - 1))
            for ti in range(2):
                nc.vector.tensor_add(
                    V[:, 2 * kcp + ti, os_ * 8:(os_ + 1) * 8, 0:DK],
                    pp[:, ti * N:(ti + 1) * N].rearrange(
                        "p (h k) -> p h k", h=8),
                    bv_sb[:, os_ * N:(os_ + 1) * N].rearrange(
                        "p (h k) -> p h k", h=8))

    if upto == "proj":
        anchor(KT[:, 0, 0:N])
        anchor(QT[:, 0, 0:N])
        anchor(V[:, 0, :, :].rearrange("p a b -> p (a b)")[:, 0:N])
        return
    # =====================================================================
    # Stage C: attention -> O^T feature-major (bf16)
    # =====================================================================
    # Attention interleaved with the out projection: after each head
    # pair (feature chunk hc) finishes, its Wo contribution accumulates
    # into r1 (SBUF, DVE adds). The extra in-phase PE work raises PE
    # density during the ACT-bound softmax steady state.
    OT = pool.tile([P, DC, TQ], BF, tag="chD")
    r1 = pool.tile([P, DC, TQ], F32, tag="chA")
    woA = pool.tile([P, DC, 512], BF, tag="w", bufs=2)
    nc.sync.dma_start(woA[:], d["wo"][:, 0, :, :])
    woB = pool.tile([P, DC, 512], BF, tag="w", bufs=2)
    nc.sync.dma_start(woB[:], d["wo"][:, 1, :, :])
    wog = (woA, woB)
    for hc in range(DC):
        for hi in range(2):
            h = 2 * hc + hi
            hp = hi * DK
            o_ps = ps.tile([P, N2], F32, tag="psB", bufs=2)
            for kc in range(KC):
                s_ps = ps.tile([P, N2], F32, tag="psA", bufs=2)
                k_sl = KT[hp:hp + DK, hc, kc * P:(kc + 1) * P]
                nc.tensor.matmul(s_ps[:, 0:N], k_sl, QT[hp:hp + DK, hc, 0:N])
                nc.tensor.matmul(s_ps[:, N:N2], k_sl,
                                 QT[hp:hp + DK, hc, N:N2])
                e = pool.tile([P, N2], BF, tag="e", bufs=2)
                nc.scalar.activation(e[:], s_ps[:], AF.Exp,
                                     bias=mask_sb[:, kc:kc + 1], scale=0.125)
                nc.tensor.matmul(o_ps[0:DK + 1, 0:N], V[:, kc, h, :],
                                 e[:, 0:N],
                                 start=(kc == 0), stop=(kc == KC - 1))
                nc.tensor.matmul(o_ps[0:DK + 1, N:N2], V[:, kc, h, :],
                                 e[:, N:N2],
                                 start=(kc == 0), stop=(kc == KC - 1))
            recip = pool.tile([1, N2], F32, tag="recip", bufs=1)
            nc.vector.reciprocal(recip[:], o_ps[DK:DK + 1, :])
            r_ps = ps.tile([P, N2], F32, tag="psA", bufs=2)
            nc.tensor.matmul(r_ps[0:DK, 0:N], ones_row_f32[0:1, 0:DK],
                             recip[0:1, 0:N])
            nc.tensor.matmul(r_ps[0:DK, N:N2], ones_row_f32[0:1, 0:DK],
                             recip[0:1, N:N2])
            rbc = pool.tile([DK, N2], F32, tag="rbc", bufs=1)
            nc.vector.tensor_copy(rbc[:], r_ps[0:DK, :])
            nc.vector.tensor_mul(OT[hp:hp + DK, hc, :], o_ps[0:DK, :],
                                 rbc[:])
        # out-projection contribution of feature chunk hc
        for oc in range(DC):
            w_sl = wog[oc // 4][:, hc, (oc % 4) * P:((oc % 4) + 1) * P]
            pp = ps.tile([P, N2], F32, tag="psA", bufs=2)
            nc.tensor.matmul(pp[:, 0:N], w_sl, OT[:, hc, 0:N])
            nc.tensor.matmul(pp[:, N:N2], w_sl, OT[:, hc, N:N2])
            if hc == 0:
                xq2 = pool.tile([P, TQ], F32, tag="xq2", bufs=1)
                nc.sync.dma_start(
                    xq2[:],
                    d["xqT"].rearrange("(c p) t -> p c t", p=P)[:, oc, :])
                nc.vector.scalar_tensor_tensor(
                    out=r1[:, oc, :], in0=pp[:],
                    scalar=bo_sb[:, oc:oc + 1], in1=xq2[:],
                    op0=OP.add, op1=OP.add)
            else:
                nc.vector.tensor_add(r1[:, oc, :], r1[:, oc, :], pp[:])

    if upto == "attn":
        anchor(OT[:, 0, 0:N])
        return

    # =====================================================================
    # Stage E: LN2 -> h2 (bf16)
    # =====================================================================
    RS2 = pool.tile([P, TQ], F32, tag="RSx")
    M22 = pool.tile([P, TQ], F32, tag="M2x")
    _ln_feature(nc, pool, ps, r1, TQ, ones_col_f32, ones_col_bf,
                ones_row_f32, eps_bias, RS2, M22)
    h2 = pool.tile([P, DC, TQ], BF, tag="chC")
    for c in range(DC):
        nc.vector.tensor_mul(h2[:, c, :], r1[:, c, :], RS2[:])
        nc.vector.tensor_add(h2[:, c, :], h2[:, c, :], M22[:])

    if upto == "ln2":
        anchor(h2[:, 0, 0:N])
        return
    # =====================================================================
    # Stage F: FFN, one query-tile (512 tokens) at a time
    # =====================================================================
    for qt in range(NQ):
        g = pool.tile([P, FC, N], BF, tag="chB")   # gelu(fc1) for this qt
        for fcg in range(8):
            w1g = pool.tile([P, DC, 512], BF, tag="w", bufs=2)
            nc.sync.dma_start(w1g[:], d["w1"][:, fcg, :, :])
            for fip in range(2):
                fc = fcg * 4 + fip * 2
                pp = ps.tile([P, N2], F32, tag="psA", bufs=2)
                for c in range(DC):
                    h_sl = h2[:, c, qt * N:(qt + 1) * N]
                    nc.tensor.matmul(
                        pp[:, 0:N],
                        w1g[:, c, (2 * fip) * P:(2 * fip + 1) * P], h_sl,
                        start=(c == 0), stop=(c == DC - 1))
                    nc.tensor.matmul(
                        pp[:, N:N2],
                        w1g[:, c, (2 * fip + 1) * P:(2 * fip + 2) * P], h_sl,
                        start=(c == 0), stop=(c == DC - 1))
                nc.scalar.activation(g[:, fc, :], pp[:, 0:N], AF.Gelu,
                                     bias=b1_sb[:, fc:fc + 1], scale=1.0)
                nc.scalar.activation(g[:, fc + 1, :], pp[:, N:N2], AF.Gelu,
                                     bias=b1_sb[:, fc + 1:fc + 2], scale=1.0)
        for oc in range(DC):
            w2g = pool.tile([P, FC, P], BF, tag="w", bufs=2)
            nc.sync.dma_start(w2g[:], d["w2"][:, oc, :, :])
            pp = ps.tile([P, N2], F32, tag="psB", bufs=2)
            for f in range(FC):
                nc.tensor.matmul(pp[:, 0:N], w2g[:, f, :], g[:, f, :],
                                 start=(f == 0), stop=(f == FC - 1))
            ot = pool.tile([P, N], F32, tag="out", bufs=2)
            nc.vector.scalar_tensor_tensor(
                out=ot[:], in0=pp[:, 0:N], scalar=b2_sb[:, oc:oc + 1],
                in1=r1[:, oc, qt * N:(qt + 1) * N],
                op0=OP.add, op1=OP.add)
            nc.sync.dma_start(
                d["outT"][oc * P:(oc + 1) * P, qt * N:(qt + 1) * N], ot[:])


def build_program(repeat=1, upto="full"):
    nc = bacc.Bacc("TRN2", target_bir_lowering=False)
    d = {}
    d["xkvT"] = nc.declare_dram_parameter("xkvT", [D, TKV], BF, isOutput=False)[:]
    d["xqT"] = nc.declare_dram_parameter("xqT", [D, TQ], F32, isOutput=False)[:]
    for w, sh in [("wq", [P, 2, DC, 512]), ("wk", [P, 2, DC, 512]),
                  ("wv", [P, 2, DC, 512]), ("wo", [P, 2, DC, 512]),
                  ("w1", [P, 8, DC, 512]), ("w2", [P, DC, FC, P])]:
        d[w] = nc.declare_dram_parameter(w, sh, BF, isOutput=False)[:]
    for b, sh in [("bq_c", [P, DC]), ("bk_c", [P, DC]), ("bo_c", [P, DC]),
                  ("b2_c", [P, DC]), ("b1_c", [P, FC]), ("maskb_c", [P, KC]),
                  ("bvb", [P, D])]:
        d[b] = nc.declare_dram_parameter(b, sh, F32, isOutput=False)[:]
    d["outT"] = nc.declare_dram_parameter("outT", [D, TQ], F32, isOutput=True)[:]

    with tile.TileContext(nc) as tc:
        for _ in range(repeat):
            build_body(tc, d, upto)
    nc.compile()
    return nc


_PROG_CACHE = {}


def _get_prog(repeat=1, upto="full"):
    key = (repeat, upto)
    if key not in _PROG_CACHE:
        _PROG_CACHE[key] = build_program(repeat, upto)
    return _PROG_CACHE[key]


def _tile_w(Wm, ngrp, gsz):
    """[1024-in, ngrp*gsz-out] -> [128, ngrp, in-chunks, gsz] host tiling."""
    ic = Wm.shape[0] // P
    return np.ascontiguousarray(
        Wm.reshape(ic, P, ngrp, gsz).transpose(1, 2, 0, 3))


def make_in_maps(x, mask, Wq, bq, Wk, bk, Wv, bv, Wo, bo, W1, b1, W2, b2,
                 ln1_g, ln1_b, ln2_g, ln2_b):
    x = np.asarray(x, np.float32)
    bf = ml_dtypes.bfloat16

    def fold(Wm, bm):
        Wf = np.asarray(ln1_g, np.float64)[:, None] * np.asarray(Wm, np.float64)
        bfold = (np.asarray(ln1_b, np.float64) @ np.asarray(Wm, np.float64)
                 + np.asarray(bm, np.float64))
        return Wf.astype(np.float32), bfold.astype(np.float32)

    wq_f, bq_f = fold(Wq, bq)
    wk_f, bk_f = fold(Wk, bk)
    wv_f, bv_f = fold(Wv, bv)
    w1_f = (np.asarray(ln2_g, np.float64)[:, None]
            * np.asarray(W1, np.float64)).astype(np.float32)
    b1_f = (np.asarray(ln2_b, np.float64) @ np.asarray(W1, np.float64)
            + np.asarray(b1, np.float64)).astype(np.float32)

    def cols(v, k):
        return np.ascontiguousarray(np.asarray(v, np.float32).reshape(k, P).T)

    shared = {
        "wq": _tile_w(wq_f, 2, 512).astype(bf),
        "wk": _tile_w(wk_f, 2, 512).astype(bf),
        "wv": _tile_w(wv_f, 2, 512).astype(bf),
        "wo": _tile_w(np.asarray(Wo, np.float32), 2, 512).astype(bf),
        "w1": _tile_w(w1_f, 8, 512).astype(bf),
        "w2": _tile_w(np.asarray(W2, np.float32), 8, 128).astype(bf),
        "bq_c": cols(bq_f, DC), "bk_c": cols(bk_f, DC),
        "bo_c": cols(np.asarray(bo, np.float32), DC),
        "b2_c": cols(np.asarray(b2, np.float32), DC),
        "b1_c": cols(b1_f, FC),
        "bvb": np.ascontiguousarray(
            np.broadcast_to(bv_f, (P, D)).astype(np.float32)),
    }
    in_maps = []
    mask = np.asarray(mask)
    for c in range(8):
        b, q = c // 2, c % 2
        xT = np.ascontiguousarray(x[b].T)                     # [D, TKV]
        mb = np.where(mask[b, 0, 0, :] == 0, np.float32(-1e9),
                      np.float32(0.0)).astype(np.float32)
        m = dict(shared)
        m["xkvT"] = xT.astype(bf)
        m["xqT"] = np.ascontiguousarray(xT[:, q * TQ:(q + 1) * TQ])
        m["maskb_c"] = np.ascontiguousarray(mb.reshape(KC, P).T)
        in_maps.append(m)
    return in_maps


def gather_out(results):
    out = np.empty((4, 2048, 1024), np.float32)
    for c in range(8):
        b, q = c // 2, c % 2
        out[b, q * TQ:(q + 1) * TQ, :] = results[c]["outT"].T
    return out


def kernel(**inputs):
    nc = _get_prog(1)
    in_maps = make_in_maps(**inputs)
    res = run_bass_kernel_spmd(nc, in_maps, list(range(8)))
    return gather_out(res.results)


# revision 20
# speedup vs baseline: 892.0221x; 1.4184x over previous
"""Trainium2 Bass kernel for a pre-norm transformer encoder layer.

Problem shapes (hardcoded): x [4, 2048, 1024], 16 heads x 64, d_ff 4096.

Sharding: 8 cores = (batch b, query-half q): core c -> b = c//2, q-rows
[q*1024, (q+1)*1024). Each core computes K/V for its batch's full 2048
tokens (12% duplicated projection work) and everything else for its own
1024 query tokens. No collectives.

On-chip layout is feature-major throughout: activations are stored
transposed ([d on partitions, tokens on free]), which every matmul here
wants (contraction over partitions); the host passes x pre-transposed
and transposes the output back. LayerNorm statistics are computed with
ones-vector matmuls on the tensor engine (partition-dim reductions) and
per-token rows are broadcast across partitions with a rank-1 matmul
(ACT Sqrt + DVE reciprocal for 1/std). LN gain/bias are folded
into the following projection weights on the host. Softmax runs without
max-subtraction (scores here are O(8), exp is safe in fp32); the
attention mask folds into the Exp activation's per-partition bias; the
softmax denominator comes free as a 65th ones-column appended to V in
the P@V matmul, normalized once per head over all 1024 queries.

PSUM scheme: two tags of [128, 1024] f32 x 2 bufs = 8 banks. Matmuls
write 512-wide halves (bank limit); ACT/DVE consume the full 1024-wide
tile in one instruction, halving ACT/DVE instruction counts and per-
instruction overhead. Matmul pairs writing both halves share their
stationary operand back-to-back.

Matmul inputs are bf16 (fp32 PSUM accumulation); the residual stream
stays fp32.

SBUF is tight (203KB/partition), so the big tensors share tag "chains"
(same pool tag -> same slots, the Tile scheduler serializes reuse):
  chA: h1 -> r1         chB: xkv -> KT -> g_qt      chC: xq -> V -> h2
  chD: h1q -> OT        chE: QT
Weights stream through one 2-buffered 8KB tag in host-pre-tiled layouts
(long DMA lines).
"""

import numpy as np
import ml_dtypes

import concourse.bass as bass
import concourse.bacc as bacc
import concourse.tile as tile
from concourse import mybir
from concourse.bass_utils import run_bass_kernel_spmd

AF = mybir.ActivationFunctionType
OP = mybir.AluOpType
BF = mybir.dt.bfloat16
F32 = mybir.dt.float32

P = 128
D = 1024
DC = D // P            # 8 d-chunks
TKV = 2048             # kv tokens per core (= S of its batch)
TQ = 1024              # query tokens per core
NKV = TKV // 512       # 4
NQ = TQ // 512         # 2
KC = TKV // P          # 16 key chunks
H = 16
DK = 64
F = 4096
FC = F // P            # 32 ff chunks
EPS = 1e-5
N = 512                # matmul moving free dim / psum bank (fp32)
N2 = 2 * N             # wide tile: 2 psum banks


def _ln_feature(nc, pool, ps, src, ntok, ones_sum, ones_bf, ones_row,
                eps_bias, RS, M2):
    """LayerNorm over the partition (d) axis of feature-major src
    [P, DC, ntok]: per 1024-token slice compute sum / sum-of-squares via
    ones-matmuls (512-wide halves), derive rs = 1/sqrt(var+eps) and
    m2 = -mu*rs rows, and broadcast them across partitions into RS/M2.
    ones_sum matches src dtype; squares are bf16 (ones_bf)."""
    for t in range(ntok // N2):
        sum_ps = ps.tile([P, N2], F32, tag="psA", bufs=2)
        sumsq_ps = ps.tile([P, N2], F32, tag="psB", bufs=2)
        for c in range(DC):
            sl = src[:, c, t * N2:(t + 1) * N2]
            nc.tensor.matmul(sum_ps[0:1, 0:N], ones_sum[:], sl[:, 0:N],
                             start=(c == 0), stop=(c == DC - 1))
            nc.tensor.matmul(sum_ps[0:1, N:N2], ones_sum[:], sl[:, N:N2],
                             start=(c == 0), stop=(c == DC - 1))
            sq = pool.tile([P, N2], BF, tag="sq", bufs=3)
            nc.scalar.activation(sq[:], sl, AF.Square)
            nc.tensor.matmul(sumsq_ps[0:1, 0:N], ones_bf[:], sq[:, 0:N],
                             start=(c == 0), stop=(c == DC - 1))
            nc.tensor.matmul(sumsq_ps[0:1, N:N2], ones_bf[:], sq[:, N:N2],
                             start=(c == 0), stop=(c == DC - 1))
        mu = pool.tile([1, N2], F32, tag="lnrow", bufs=4)
        nc.vector.tensor_scalar_mul(mu[:], sum_ps[0:1, :], 1.0 / D)
        musq = pool.tile([1, N2], F32, tag="lnrow", bufs=4)
        nc.vector.tensor_mul(musq[:], mu[:], mu[:])
        var = pool.tile([1, N2], F32, tag="lnrow", bufs=4)
        nc.vector.scalar_tensor_tensor(
            out=var[:], in0=sumsq_ps[0:1, :], scalar=1.0 / D, in1=musq[:],
            op0=OP.mult, op1=OP.subtract)
        std = pool.tile([1, N2], F32, tag="lnrow", bufs=4)
        nc.scalar.activation(std[:], var[:], AF.Sqrt, bias=eps_bias,
                             scale=1.0)
        # rows in RS/M2 dtype so the GpSimd partition broadcast (which
        # requires matching dtypes) can replace the old rank-1 matmul
        # broadcast; that matmul made the PE wait out the 6.5us DVE
        # reciprocal in its in-order stream.
        rs = pool.tile([1, N2], RS.dtype, tag="lnrow", bufs=4)
        m2 = pool.tile([1, N2], RS.dtype, tag="lnrow", bufs=4)
        with nc.allow_low_precision(
                reason="LN rows cast to the RS/M2 dtype (bf16 for the kv "
                       "LN) before broadcast - same precision as the old "
                       "psum->bf16 broadcast copy"):
            nc.vector.reciprocal(rs[:], std[:])
            nc.vector.scalar_tensor_tensor(
                out=m2[:], in0=mu[:], scalar=-1.0, in1=rs[:],
                op0=OP.mult, op1=OP.mult)
        nc.gpsimd.partition_broadcast(RS[:, t * N2:(t + 1) * N2],
                                      rs[0:1, :], channels=P)
        nc.gpsimd.partition_broadcast(M2[:, t * N2:(t + 1) * N2],
                                      m2[0:1, :], channels=P)


def build_body(tc, d, upto="full"):
    nc = tc.nc
    with tc.tile_pool(name="sb", bufs=1) as pool, \
         tc.tile_pool(name="ps", bufs=2, space="PSUM") as ps:
        _build_inner(nc, pool, ps, d, upto)


def _build_inner(nc, pool, ps, d, upto="full"):
    def anchor(t):
        """DMA a slice out to keep the truncated pipeline live."""
        nc.gpsimd.dma_start(out=d["outT"][0:P, 0:N], in_=t)
    # --- constants ---
    ones_col_bf = pool.tile([P, 1], BF, tag="c_onesb")
    nc.vector.memset(ones_col_bf[:], 1.0)
    ones_col_f32 = pool.tile([P, 1], F32, tag="c_onesf")
    nc.vector.memset(ones_col_f32[:], 1.0)
    ones_row_f32 = pool.tile([1, P], F32, tag="c_onesr")
    nc.vector.memset(ones_row_f32[:], 1.0)
    eps_bias = pool.tile([1, 1], F32, tag="c_eps")
    nc.vector.memset(eps_bias[:], EPS)

    bias_c = {}
    for nm, w in [("bq_c", DC), ("bk_c", DC), ("bo_c", DC), ("b2_c", DC),
                  ("b1_c", FC), ("maskb_c", KC)]:
        t = pool.tile([P, w], F32, tag=f"c_{nm}")
        nc.sync.dma_start(t[:], d[nm][:])
        bias_c[nm] = t
    bq_sb, bk_sb, bo_sb = bias_c["bq_c"], bias_c["bk_c"], bias_c["bo_c"]
    b2_sb, b1_sb, mask_sb = bias_c["b2_c"], bias_c["b1_c"], bias_c["maskb_c"]
    bv_sb = pool.tile([P, D], F32, tag="c_bv")
    nc.sync.dma_start(bv_sb[:], d["bvb"][:])

    # =====================================================================
    # Stage A: LN1 over kv tokens (2048) and q tokens (1024), apply
    # =====================================================================
    xkv = pool.tile([P, DC, TKV], BF, tag="chB")
    nc.sync.dma_start(xkv[:], d["xkvT"].rearrange("(c p) t -> p c t", p=P))
    xq_sb = pool.tile([P, DC, TQ], F32, tag="chC")
    nc.sync.dma_start(xq_sb[:], d["xqT"].rearrange("(c p) t -> p c t", p=P))

    if upto == "load":
        anchor(xkv[:, 0, 0:N])
        anchor(xq_sb[:, 0, 0:N])
        return
    RS1 = pool.tile([P, TKV], BF, tag="RS1")
    M21 = pool.tile([P, TKV], BF, tag="M21")
    _ln_feature(nc, pool, ps, xkv, TKV, ones_col_bf, ones_col_bf,
                ones_row_f32, eps_bias, RS1, M21)
    if upto == "lnkv":
        anchor(RS1[:, 0:N])
        anchor(M21[:, 0:N])
        return
    RSq = pool.tile([P, TQ], F32, tag="RSx")
    M2q = pool.tile([P, TQ], F32, tag="M2x")
    _ln_feature(nc, pool, ps, xq_sb, TQ, ones_col_f32, ones_col_bf,
                ones_row_f32, eps_bias, RSq, M2q)

    # apply LN (in-place second op) -> h1T, h1qT (bf16)
    h1 = pool.tile([P, DC, TKV], BF, tag="chA")
    h1q = pool.tile([P, DC, TQ], BF, tag="chD")
    for c in range(DC):
        nc.vector.tensor_mul(h1[:, c, :], xkv[:, c, :], RS1[:])
        nc.vector.tensor_add(h1[:, c, :], h1[:, c, :], M21[:])
        nc.vector.tensor_mul(h1q[:, c, :], xq_sb[:, c, :], RSq[:])
        nc.vector.tensor_add(h1q[:, c, :], h1q[:, c, :], M2q[:])

    if upto == "ln":
        anchor(h1[:, 0, 0:N])
        return
    # =====================================================================
    # Stage B: Q/K/V projections (weights streamed, host-pre-tiled)
    # =====================================================================
    KT = pool.tile([P, DC, TKV], BF, tag="chB")     # K^T feature-major
    QT = pool.tile([P, DC, TQ], BF, tag="chE")      # Q^T feature-major
    V = pool.tile([P, KC, H, DK + 1], BF, tag="chC")  # token-major V + ones
    nc.vector.memset(V[:, :, :, DK:DK + 1], 1.0)

    for ocg in range(2):
        wkg = pool.tile([P, DC, 512], BF, tag="w", bufs=2)
        nc.sync.dma_start(wkg[:], d["wk"][:, ocg, :, :])
        for oci in range(4):
            oc = ocg * 4 + oci
            for tp in range(NKV // 2):
                pp = ps.tile([P, N2], F32, tag="psA", bufs=2)
                for c in range(DC):
                    w_sl = wkg[:, c, oci * P:(oci + 1) * P]
                    nc.tensor.matmul(pp[:, 0:N], w_sl,
                                     h1[:, c, tp * N2:tp * N2 + N],
                                     start=(c == 0), stop=(c == DC - 1))
                    nc.tensor.matmul(pp[:, N:N2], w_sl,
                                     h1[:, c, tp * N2 + N:(tp + 1) * N2],
                                     start=(c == 0), stop=(c == DC - 1))
                nc.scalar.activation(KT[:, oc, tp * N2:(tp + 1) * N2], pp[:],
                                     AF.Identity, bias=bk_sb[:, oc:oc + 1],
                                     scale=1.0)
    for ocg in range(2):
        wqg = pool.tile([P, DC, 512], BF, tag="w", bufs=2)
        nc.sync.dma_start(wqg[:], d["wq"][:, ocg, :, :])
        for oci in range(4):
            oc = ocg * 4 + oci
            pp = ps.tile([P, N2], F32, tag="psA", bufs=2)
            for c in range(DC):
                w_sl = wqg[:, c, oci * P:(oci + 1) * P]
                nc.tensor.matmul(pp[:, 0:N], w_sl, h1q[:, c, 0:N],
                                 start=(c == 0), stop=(c == DC - 1))
                nc.tensor.matmul(pp[:, N:N2], w_sl, h1q[:, c, N:N2],
                                 start=(c == 0), stop=(c == DC - 1))
            nc.scalar.activation(QT[:, oc, :], pp[:],
                                 AF.Identity, bias=bq_sb[:, oc:oc + 1],
                                 scale=1.0)
    for os_ in range(2):
        wvg = pool.tile([P, DC, 512], BF, tag="w", bufs=2)
        nc.sync.dma_start(wvg[:], d["wv"][:, os_, :, :])
        for kcp in range(KC // 2):
            pp = ps.tile([P, N2], F32, tag="psB", bufs=2)
            for c in range(DC):
                nc.tensor.matmul(
                    pp[:, 0:N], h1[:, c, (2 * kcp) * P:(2 * kcp + 1) * P],
                    wvg[:, c, :], start=(c == 0), stop=(c == DC - 1))
                nc.tensor.matmul(
                    pp[:, N:N2], h1[:, c, (2 * kcp + 1) * P:(2 * kcp + 2) * P],
                    wvg[:, c, :], start=(c == 0), stop=(c == DC - 1))
            for ti in range(2):
                nc.vector.tensor_add(
                    V[:, 2 * kcp + ti, os_ * 8:(os_ + 1) * 8, 0:DK],
                    pp[:, ti * N:(ti + 1) * N].rearrange(
                        "p (h k) -> p h k", h=8),
                    bv_sb[:, os_ * N:(os_ + 1) * N].rearrange(
                        "p (h k) -> p h k", h=8))

    if upto == "proj":
        anchor(KT[:, 0, 0:N])
        anchor(QT[:, 0, 0:N])
        anchor(V[:, 0, :, :].rearrange("p a b -> p (a b)")[:, 0:N])
        return
    # =====================================================================
    # Stage C: attention -> O^T feature-major (bf16)
    # =====================================================================
    OT = pool.tile([P, DC, TQ], BF, tag="chD")
    for h in range(H):
        hp = (h % 2) * DK
        hc = h // 2
        o_ps = ps.tile([P, N2], F32, tag="psB", bufs=2)
        prev_s = None
        for kc in range(KC):
            s_ps = ps.tile([P, N2], F32, tag="psA", bufs=2)
            k_sl = KT[hp:hp + DK, hc, kc * P:(kc + 1) * P]
            nc.tensor.matmul(s_ps[:, 0:N], k_sl, QT[hp:hp + DK, hc, 0:N])
            nc.tensor.matmul(s_ps[:, N:N2], k_sl, QT[hp:hp + DK, hc, N:N2])
            e = pool.tile([P, N2], BF, tag="e", bufs=2)
            nc.scalar.activation(e[:], s_ps[:], AF.Exp,
                                 bias=mask_sb[:, kc:kc + 1], scale=0.125)
            nc.tensor.matmul(o_ps[0:DK + 1, 0:N], V[:, kc, h, :], e[:, 0:N],
                             start=(kc == 0), stop=(kc == KC - 1))
            nc.tensor.matmul(o_ps[0:DK + 1, N:N2], V[:, kc, h, :],
                             e[:, N:N2],
                             start=(kc == 0), stop=(kc == KC - 1))
            prev_s = s_ps
        # Normalize tail entirely off the PE: the old rank-1 broadcast
        # matmuls made the in-order PE stream wait out the 6.5us DVE
        # reciprocal once per head (~112us of PE idle). GpSimd does the
        # partition broadcast instead.
        recip = pool.tile([1, N2], F32, tag="recip", bufs=1)
        nc.vector.reciprocal(recip[:], o_ps[DK:DK + 1, :])
        rbc = pool.tile([DK, N2], F32, tag="rbc", bufs=1)
        nc.gpsimd.partition_broadcast(rbc[:], recip[:], channels=DK)
        nc.vector.tensor_mul(OT[hp:hp + DK, hc, :], o_ps[0:DK, :], rbc[:])

    if upto == "attn":
        anchor(OT[:, 0, 0:N])
        return
    # =====================================================================
    # Stage D: out projection + residual -> r1 (fp32)
    # =====================================================================
    r1 = pool.tile([P, DC, TQ], F32, tag="chA")
    for ocg in range(2):
        wog = pool.tile([P, DC, 512], BF, tag="w", bufs=2)
        nc.sync.dma_start(wog[:], d["wo"][:, ocg, :, :])
        for oci in range(4):
            oc = ocg * 4 + oci
            xq2 = pool.tile([P, TQ], F32, tag="xq2", bufs=1)
            nc.sync.dma_start(
                xq2[:],
                d["xqT"].rearrange("(c p) t -> p c t", p=P)[:, oc, :])
            pp = ps.tile([P, N2], F32, tag="psA", bufs=2)
            for c in range(DC):
                w_sl = wog[:, c, oci * P:(oci + 1) * P]
                nc.tensor.matmul(pp[:, 0:N], w_sl, OT[:, c, 0:N],
                                 start=(c == 0), stop=(c == DC - 1))
                nc.tensor.matmul(pp[:, N:N2], w_sl, OT[:, c, N:N2],
                                 start=(c == 0), stop=(c == DC - 1))
            nc.vector.scalar_tensor_tensor(
                out=r1[:, oc, :], in0=pp[:],
                scalar=bo_sb[:, oc:oc + 1], in1=xq2[:],
                op0=OP.add, op1=OP.add)

    # =====================================================================
    # Stage E: LN2 -> h2 (bf16)
    # =====================================================================
    RS2 = pool.tile([P, TQ], F32, tag="RSx")
    M22 = pool.tile([P, TQ], F32, tag="M2x")
    _ln_feature(nc, pool, ps, r1, TQ, ones_col_f32, ones_col_bf,
                ones_row_f32, eps_bias, RS2, M22)
    h2 = pool.tile([P, DC, TQ], BF, tag="chC")
    for c in range(DC):
        nc.vector.tensor_mul(h2[:, c, :], r1[:, c, :], RS2[:])
        nc.vector.tensor_add(h2[:, c, :], h2[:, c, :], M22[:])

    if upto == "ln2":
        anchor(h2[:, 0, 0:N])
        return
    # =====================================================================
    # Stage F: FFN, one query-tile (512 tokens) at a time
    # =====================================================================
    for qt in range(NQ):
        g = pool.tile([P, FC, N], BF, tag="chB")   # gelu(fc1) for this qt
        for fcg in range(8):
            w1g = pool.tile([P, DC, 512], BF, tag="w", bufs=2)
            nc.sync.dma_start(w1g[:], d["w1"][:, fcg, :, :])
            for fip in range(2):
                fc = fcg * 4 + fip * 2
                pp = ps.tile([P, N2], F32, tag="psA", bufs=2)
                for c in range(DC):
                    h_sl = h2[:, c, qt * N:(qt + 1) * N]
                    nc.tensor.matmul(
                        pp[:, 0:N],
                        w1g[:, c, (2 * fip) * P:(2 * fip + 1) * P], h_sl,
                        start=(c == 0), stop=(c == DC - 1))
                    nc.tensor.matmul(
                        pp[:, N:N2],
                        w1g[:, c, (2 * fip + 1) * P:(2 * fip + 2) * P], h_sl,
                        start=(c == 0), stop=(c == DC - 1))
                nc.scalar.activation(g[:, fc, :], pp[:, 0:N], AF.Gelu,
                                     bias=b1_sb[:, fc:fc + 1], scale=1.0)
                nc.scalar.activation(g[:, fc + 1, :], pp[:, N:N2], AF.Gelu,
                                     bias=b1_sb[:, fc + 1:fc + 2], scale=1.0)
        for oc in range(DC):
            w2g = pool.tile([P, FC, P], BF, tag="w", bufs=2)
            nc.sync.dma_start(w2g[:], d["w2"][:, oc, :, :])
            pp = ps.tile([P, N2], F32, tag="psB", bufs=2)
            for f in range(FC):
                nc.tensor.matmul(pp[:, 0:N], w2g[:, f, :], g[:, f, :],
                                 start=(f == 0), stop=(f == FC - 1))
            ot = pool.tile([P, N], F32, tag="out", bufs=2)
            nc.vector.scalar_tensor_tensor(
                out=ot[:], in0=pp[:, 0:N], scalar=b2_sb[:, oc:oc + 1],
                in1=r1[:, oc, qt * N:(qt + 1) * N],
                op0=OP.add, op1=OP.add)
            nc.sync.dma_start(
                d["outT"][oc * P:(oc + 1) * P, qt * N:(qt + 1) * N], ot[:])


def build_program(repeat=1, upto="full"):
    nc = bacc.Bacc("TRN2", target_bir_lowering=False)
    d = {}
    d["xkvT"] = nc.declare_dram_parameter("xkvT", [D, TKV], BF, isOutput=False)[:]
    d["xqT"] = nc.declare_dram_parameter("xqT", [D, TQ], F32, isOutput=False)[:]
    for w, sh in [("wq", [P, 2, DC, 512]), ("wk", [P, 2, DC, 512]),
                  ("wv", [P, 2, DC, 512]), ("wo", [P, 2, DC, 512]),
                  ("w1", [P, 8, DC, 512]), ("w2", [P, DC, FC, P])]:
        d[w] = nc.declare_dram_parameter(w, sh, BF, isOutput=False)[:]
    for b, sh in [("bq_c", [P, DC]), ("bk_c", [P, DC]), ("bo_c", [P, DC]),
                  ("b2_c", [P, DC]), ("b1_c", [P, FC]), ("maskb_c", [P, KC]),
                  ("bvb", [P, D])]:
        d[b] = nc.declare_dram_parameter(b, sh, F32, isOutput=False)[:]
    d["outT"] = nc.declare_dram_parameter("outT", [D, TQ], F32, isOutput=True)[:]

    with tile.TileContext(nc) as tc:
        for _ in range(repeat):
            build_body(tc, d, upto)
    nc.compile()
    return nc


_PROG_CACHE = {}


def _get_prog(repeat=1, upto="full"):
    key = (repeat, upto)
    if key not in _PROG_CACHE:
        _PROG_CACHE[key] = build_program(repeat, upto)
    return _PROG_CACHE[key]


def _tile_w(Wm, ngrp, gsz):
    """[1024-in, ngrp*gsz-out] -> [128, ngrp, in-chunks, gsz] host tiling."""
    ic = Wm.shape[0] // P
    return np.ascontiguousarray(
        Wm.reshape(ic, P, ngrp, gsz).transpose(1, 2, 0, 3))


def make_in_maps(x, mask, Wq, bq, Wk, bk, Wv, bv, Wo, bo, W1, b1, W2, b2,
                 ln1_g, ln1_b, ln2_g, ln2_b):
    x = np.asarray(x, np.float32)
    bf = ml_dtypes.bfloat16

    def fold(Wm, bm):
        Wf = np.asarray(ln1_g, np.float64)[:, None] * np.asarray(Wm, np.float64)
        bfold = (np.asarray(ln1_b, np.float64) @ np.asarray(Wm, np.float64)
                 + np.asarray(bm, np.float64))
        return Wf.astype(np.float32), bfold.astype(np.float32)

    wq_f, bq_f = fold(Wq, bq)
    wk_f, bk_f = fold(Wk, bk)
    wv_f, bv_f = fold(Wv, bv)
    w1_f = (np.asarray(ln2_g, np.float64)[:, None]
            * np.asarray(W1, np.float64)).astype(np.float32)
    b1_f = (np.asarray(ln2_b, np.float64) @ np.asarray(W1, np.float64)
            + np.asarray(b1, np.float64)).astype(np.float32)

    def cols(v, k):
        return np.ascontiguousarray(np.asarray(v, np.float32).reshape(k, P).T)

    shared = {
        "wq": _tile_w(wq_f, 2, 512).astype(bf),
        "wk": _tile_w(wk_f, 2, 512).astype(bf),
        "wv": _tile_w(wv_f, 2, 512).astype(bf),
        "wo": _tile_w(np.asarray(Wo, np.float32), 2, 512).astype(bf),
        "w1": _tile_w(w1_f, 8, 512).astype(bf),
        "w2": _tile_w(np.asarray(W2, np.float32), 8, 128).astype(bf),
        "bq_c": cols(bq_f, DC), "bk_c": cols(bk_f, DC),
        "bo_c": cols(np.asarray(bo, np.float32), DC),
        "b2_c": cols(np.asarray(b2, np.float32), DC),
        "b1_c": cols(b1_f, FC),
        "bvb": np.ascontiguousarray(
            np.broadcast_to(bv_f, (P, D)).astype(np.float32)),
    }
    in_maps = []
    mask = np.asarray(mask)
    for c in range(8):
        b, q = c // 2, c % 2
        xT = np.ascontiguousarray(x[b].T)                     # [D, TKV]
        mb = np.where(mask[b, 0, 0, :] == 0, np.float32(-1e9),
                      np.float32(0.0)).astype(np.float32)
        m = dict(shared)
        m["xkvT"] = xT.astype(bf)
        m["xqT"] = np.ascontiguousarray(xT[:, q * TQ:(q + 1) * TQ])
        m["maskb_c"] = np.ascontiguousarray(mb.reshape(KC, P).T)
        in_maps.append(m)
    return in_maps


def gather_out(results):
    out = np.empty((4, 2048, 1024), np.float32)
    for c in range(8):
        b, q = c // 2, c % 2
        out[b, q * TQ:(q + 1) * TQ, :] = results[c]["outT"].T
    return out


def kernel(**inputs):
    nc = _get_prog(1)
    in_maps = make_in_maps(**inputs)
    res = run_bass_kernel_spmd(nc, in_maps, list(range(8)))
    return gather_out(res.results)
